# revision 13
# baseline (speedup 1.0000x reference)
"""Bass/Trainium2 kernel for nn_CrossWindowAttention3D (8-core SPMD).

Strategy (hardcoded for shapes B_=1024, N=98, C=96, H=3, NW=512):
- Shard 1024 window-instances over 8 cores: core c owns distinct windows
  [64c, 64c+64) for both batch replicas, interleaved (b0,j),(b1,j) so the
  exp(mask+bias) table for window j is loaded once per pair.
- Host precomputes channel-major bf16 transposes of x/y, folds the qk scale
  into the Q weights, and merges mask + relative-position bias into a single
  multiplicative table emb = exp(mask + bias) so the device softmax is
  exp(qk) * emb with no additive masking pass.
- Device computes, per window: single-matmul qT/kT projections (batched
  over 4 windows; q is scattered into a zero-initialized per-head-masked
  [C, H, 4N] layout by three partition-sliced DVE casts, so qk runs as one
  full-K matmul per window), token-major v, exp on ACT,
  one multiply by emb, unnormalized head outputs + softmax denominators via
  col-tiled matmuls (an all-ones stationary broadcasts the denominators to
  all 96 channel partitions), reciprocal on DVE, one normalize multiply, and
  a channel-major projection with bias applied during the PSUM->SBUF copy.
- Output is returned channel-major [96, 12544] per core; host transposes.
"""

import sys

sys.path.insert(0, "/opt/trn_rl_repo")

import numpy as np
import ml_dtypes

import concourse.bass as bass
import concourse.tile as tile
from concourse import mybir
from concourse.vector_clock import ScopedClock
from concourse.bass_utils import run_bass_kernel_spmd

BF16 = mybir.dt.bfloat16
F32 = mybir.dt.float32
NPBF16 = ml_dtypes.bfloat16

WS = (2, 7, 7)
N = 98            # tokens per window
C = 96            # embed dim
H = 3             # heads
HD = 32           # head dim
NW = 512          # distinct windows
BWIN = 1024       # window-instances total
NCORES = 8
NI = 128          # instances per core
NJ = 64           # distinct windows per core
T = NI * N        # tokens per core = 12544
HB = H * N        # 294


# ---------------------------------------------------------------- tile patch
def _patch_tile_tail_drain():
    """This neuronxcc build rejects >1 sync wait on CTRL-class (Drain)
    instructions; split the TileContext tail-drain waits across NOPs."""
    if getattr(tile.TileContext, "_drain_patch_applied", False):
        return

    def _drain_and_barrier_split(self, tick_clock, wait_clock):
        nc = self.nc
        carrier = nc.sync.nop(nofuse=True)
        wait_clock.add_sem_waits(
            carrier.ins, ScopedClock({None: tick_clock.global_clock})
        )
        si = carrier.ins.sync_info
        waits = list(si.on_wait or []) if si is not None else []
        if len(waits) > 1:
            si.on_wait = waits[:1]
            for w in waits[1:]:
                extra = nc.sync.nop(nofuse=True)
                esi = extra.ins.sync_info
                if esi is None:
                    extra.ins.sync_info = mybir.SyncInfo(
                        on_wait=[w], on_update=[]
                    )
                else:
                    esi.on_wait = list(esi.on_wait or []) + [w]
        nc.sync.drain()
        nc.all_engine_barrier()
        assert self.sems is not None
        popped = nc._tile_sem_poison_stack.pop()
        assert popped is self._sem_poison
        nc.clear_and_free_semaphores(list(self.sems.allocated().values()))
        nc.all_engine_barrier()

    tile.TileContext._drain_and_barrier = _drain_and_barrier_split
    tile.TileContext._drain_patch_applied = True


def _split_sync_waits(nc, max_waits=1):
    """This neuronxcc build accepts at most one sync wait per instruction.
    Hoist excess waits onto same-engine NOPs inserted just before the
    instruction (the sequencer blocks on them in order; AND-semantics of
    multiple waits is preserved)."""
    ctr = 0
    for bb in nc.main_func.blocks:
        new_list = []
        changed = False
        for inst in bb.instructions:
            si = inst.sync_info
            waits = list(si.on_wait or []) if si is not None else []
            if len(waits) > max_waits:
                si.on_wait = waits[: max_waits]
                for w in waits[max_waits:]:
                    nop = mybir.InstNoOp(
                        name=f"I-waitsplit-{ctr}", ins=[], outs=[]
                    )
                    ctr += 1
                    nop.engine = inst.engine
                    nop.sync_info = mybir.SyncInfo(on_wait=[w], on_update=[])
                    new_list.append(nop)
                changed = True
            new_list.append(inst)
        if changed:
            bb.instructions = new_list


# ------------------------------------------------------------- host helpers
def _relative_position_index():
    ws = WS
    coords = np.stack(
        np.meshgrid(
            np.arange(ws[0]), np.arange(ws[1]), np.arange(ws[2]), indexing="ij"
        )
    )
    cf = coords.reshape(3, -1)
    rel = cf[:, :, None] - cf[:, None, :]
    rel = rel.transpose(1, 2, 0).astype(np.int64)
    rel[..., 0] += ws[0] - 1
    rel[..., 1] += ws[1] - 1
    rel[..., 2] += ws[2] - 1
    rel[..., 0] *= (2 * ws[1] - 1) * (2 * ws[2] - 1)
    rel[..., 1] *= 2 * ws[2] - 1
    return rel.sum(-1)  # (N, N)


REL_IDX = _relative_position_index()


# ------------------------------------------------------------ device program
_PROGRAM = None

# tiling knobs
XCH = 32          # instances per x/y SBUF chunk (4 chunks)
ECH = 8           # emb pairs per SBUF chunk (8 chunks)
G4 = 4            # instances per q/k projection batch & proj psum batch
YB = 8            # instances per output staging buffer / DMA
EMB_DVE = 96      # emb-mult columns handled by DVE; rest go to GPSIMD
RECIP_ON_DVE = False  # custom-DVE op fails walrus codegen (ISA wrong length)


def _build_program(split_waits=True, n_pairs=NI // 2):
    _patch_tile_tail_drain()
    nc = bass.Bass()

    xT = nc.declare_dram_parameter("xT", [C, T], BF16, isOutput=False)
    yT = nc.declare_dram_parameter("yT", [C, T], BF16, isOutput=False)
    emb = nc.declare_dram_parameter("emb", [N, NJ, HB], BF16, isOutput=False)
    # plain scaled q weights; the per-head mask is applied by the
    # partition-sliced PSUM->SBUF casts into zeroed [C, H, 4N] buffers,
    # so qk stays one full-K matmul per window.
    wq = nc.declare_dram_parameter("wq", [C, C], BF16, isOutput=False)
    wk = nc.declare_dram_parameter("wk", [C, C], BF16, isOutput=False)
    wv = nc.declare_dram_parameter("wv", [C, C], BF16, isOutput=False)
    pw = nc.declare_dram_parameter("pw", [C, C], BF16, isOutput=False)
    pb = nc.declare_dram_parameter("pb", [C, 1], F32, isOutput=False)
    out = nc.declare_dram_parameter("yT_out", [C, T], BF16, isOutput=True)

    from contextlib import ExitStack

    with tile.TileContext(nc) as tc:
        with ExitStack() as ctx:
            singles = ctx.enter_context(tc.tile_pool(name="singles", bufs=1))
            xt_pool = ctx.enter_context(tc.tile_pool(name="xt", bufs=2))
            yt_pool = ctx.enter_context(tc.tile_pool(name="yt", bufs=2))
            emb_pool = ctx.enter_context(tc.tile_pool(name="emb", bufs=2))
            qt_pool = ctx.enter_context(tc.tile_pool(name="qt", bufs=3))
            qsb_pool = ctx.enter_context(tc.tile_pool(name="qsb", bufs=2))
            kt_pool = ctx.enter_context(tc.tile_pool(name="kt", bufs=3))
            v_pool = ctx.enter_context(tc.tile_pool(name="v", bufs=3))
            exp_pool = ctx.enter_context(tc.tile_pool(name="exp", bufs=3))
            expT_pool = ctx.enter_context(tc.tile_pool(name="expT", bufs=4))
            r2_pool = ctx.enter_context(tc.tile_pool(name="r2", bufs=3))
            attT_pool = ctx.enter_context(tc.tile_pool(name="attT", bufs=4))
            ystage_pool = ctx.enter_context(
                tc.tile_pool(name="ystage", bufs=2)
            )
            ps_qmk = ctx.enter_context(
                tc.tile_pool(name="ps_qmk", bufs=1, space="PSUM")
            )
            ps_v = ctx.enter_context(
                tc.tile_pool(name="ps_v", bufs=1, space="PSUM")
            )
            ps_qk = ctx.enter_context(
                tc.tile_pool(name="ps_qk", bufs=2, space="PSUM")
            )
            ps_av = ctx.enter_context(
                tc.tile_pool(name="ps_av", bufs=2, space="PSUM")
            )
            ps_y = ctx.enter_context(
                tc.tile_pool(name="ps_y", bufs=1, space="PSUM")
            )
            wq_sb = singles.tile([C, C], BF16)
            nc.sync.dma_start(out=wq_sb, in_=wq[:, :])
            wk_sb = singles.tile([C, C], BF16)
            nc.sync.dma_start(out=wk_sb, in_=wk[:, :])
            wv_sb = singles.tile([C, C], BF16)
            nc.sync.dma_start(out=wv_sb, in_=wv[:, :])
            pw_sb = singles.tile([C, C], BF16)
            nc.sync.dma_start(out=pw_sb, in_=pw[:, :])
            pb_sb = singles.tile([C, 1], F32)
            nc.sync.dma_start(out=pb_sb, in_=pb[:, :])
            ones_sb = singles.tile([N, HD], BF16)
            nc.vector.memset(ones_sb, 1.0)
            # persistent masked-q staging buffers: only the per-head
            # diagonal blocks are ever (re)written by the casts below,
            # off-head blocks stay zero from this one-time memset.
            qtbufs = []
            for i in range(3):
                qb = singles.tile([C, H, G4 * N], BF16, name=f"qtb{i}")
                nc.gpsimd.memset(qb, 0.0)
                qtbufs.append(qb)

            def load_xy(ch, split=False):
                """Load x/y chunk ch; split=True issues two half DMAs so
                the first consumers start as soon as the first half lands."""
                xt_c = xt_pool.tile([C, XCH * N], BF16, name="xt_c")
                yt_c = yt_pool.tile([C, XCH * N], BF16, name="yt_c")
                base = ch * XCH * N
                hw = XCH * N // 2
                for t, src in ((xt_c, xT), (yt_c, yT)):
                    if split:
                        nc.sync.dma_start(
                            out=t[:, 0:hw], in_=src[:, base : base + hw]
                        )
                        nc.sync.dma_start(
                            out=t[:, hw:],
                            in_=src[:, base + hw : base + XCH * N],
                        )
                    else:
                        nc.sync.dma_start(
                            out=t, in_=src[:, base : base + XCH * N]
                        )
                return xt_c, yt_c

            def load_emb(ek, split=False):
                e = emb_pool.tile([N, ECH, HB], BF16, name="emb_c")
                eh = ECH // 2
                if split:
                    nc.sync.dma_start(
                        out=e[:, 0:eh, :],
                        in_=emb[:, ek * ECH : ek * ECH + eh, :],
                    )
                    nc.sync.dma_start(
                        out=e[:, eh:, :],
                        in_=emb[:, ek * ECH + eh : (ek + 1) * ECH, :],
                    )
                else:
                    nc.sync.dma_start(
                        out=e, in_=emb[:, ek * ECH : (ek + 1) * ECH, :]
                    )
                return e

            # prefetch: chunk 0 split for fast start; later chunks are
            # issued halfway through the previous chunk's pairs.
            PXCH = XCH // 2          # pairs per x/y chunk
            xy_next = load_xy(0, split=True)
            emb_next = load_emb(0, split=True)
            xt_ch = yt_ch = emb_ch = None
            qt_g = kt_g = psy = ystage = None

            for pair in range(n_pairs):
                w0 = 2 * pair
                if w0 % XCH == 0:
                    xt_ch, yt_ch = xy_next
                if (
                    pair % PXCH == PXCH // 2
                    and pair // PXCH + 1 < n_pairs // PXCH
                ):
                    xy_next = load_xy(pair // PXCH + 1)
                if pair % ECH == 0:
                    emb_ch = emb_next
                if (
                    pair % ECH == ECH // 2
                    and pair // ECH + 1 < n_pairs // ECH
                ):
                    emb_next = load_emb(pair // ECH + 1)

                if w0 % G4 == 0:
                    # q / k projections for w0 .. w0+3 (one matmul each)
                    goff = (w0 % XCH) * N
                    pq = ps_qmk.tile([C, 512], F32, name="pq", tag="pq")
                    nc.tensor.matmul(
                        out=pq[:, 0 : G4 * N],
                        lhsT=wq_sb,
                        rhs=yt_ch[:, goff : goff + G4 * N],
                    )
                    # one plain DVE cast evacuates pq; the per-head scatter
                    # into the block-diagonal qt layout is SBUF->SBUF and
                    # runs on the otherwise-idle GPSIMD engine.
                    q_sb = qsb_pool.tile([C, G4 * N], BF16, name="q_sb")
                    nc.vector.tensor_copy(q_sb, pq[:, 0 : G4 * N])
                    qt_g = qtbufs[(w0 // G4) % 3]
                    for h in range(H):
                        nc.gpsimd.tensor_copy(
                            qt_g[h * HD : (h + 1) * HD, h, :],
                            q_sb[h * HD : (h + 1) * HD, :],
                        )
                    pk = ps_qmk.tile([C, 512], F32, name="pk", tag="pk")
                    nc.tensor.matmul(
                        out=pk[:, 0 : G4 * N],
                        lhsT=wk_sb,
                        rhs=xt_ch[:, goff : goff + G4 * N],
                    )
                    kt_g = kt_pool.tile([C, G4 * N], BF16)
                    nc.vector.tensor_copy(kt_g, pk[:, 0 : G4 * N])

                # ---- v projections, batched 4 windows per psum tile/copy
                if w0 % G4 == 0:
                    pv4 = ps_v.tile([N, G4, 128], F32)
                    for j in range(G4):
                        col = ((w0 + j) % XCH) * N
                        nc.tensor.matmul(
                            out=pv4[:, j, 0:C],
                            lhsT=xt_ch[:, col : col + N],
                            rhs=wv_sb,
                        )
                    v4_sb = v_pool.tile([N, G4, C], BF16)
                    nc.vector.tensor_copy(v4_sb, pv4[:, :, 0:C])

                # ---- qk logits + exp per window
                exp_pair = exp_pool.tile([N, 2, HB], BF16)
                for k in range(2):
                    w = w0 + k
                    i4 = (w % G4) * N
                    pqk = ps_qk.tile([N, 512], F32)
                    nc.tensor.matmul(
                        out=pqk[:, 0:HB],
                        lhsT=kt_g[:, i4 : i4 + N],
                        rhs=qt_g[:, :, i4 : i4 + N],
                    )
                    nc.scalar.activation(
                        out=exp_pair[:, k, :],
                        in_=pqk[:, 0:HB],
                        func=mybir.ActivationFunctionType.Exp,
                    )

                # ---- one multiply by emb for the pair (same distinct window)
                # column-split between DVE and GPSIMD to balance engine load
                pj = pair % ECH
                expT = expT_pool.tile([N, 2, HB], BF16)
                c0 = EMB_DVE
                emb_lo = emb_ch[:, pj : pj + 1, 0:c0].broadcast_to((N, 2, c0))
                nc.vector.tensor_tensor(
                    out=expT[:, :, 0:c0],
                    in0=exp_pair[:, :, 0:c0],
                    in1=emb_lo,
                    op=mybir.AluOpType.mult,
                )
                emb_hi = emb_ch[:, pj : pj + 1, c0:HB].broadcast_to(
                    (N, 2, HB - c0)
                )
                nc.gpsimd.tensor_tensor(
                    out=expT[:, :, c0:HB],
                    in0=exp_pair[:, :, c0:HB],
                    in1=emb_hi,
                    op=mybir.AluOpType.mult,
                )

                # ---- denominators: one [C, 4N] psum tile per 4 windows
                if pair % 2 == 0:
                    pdbc = ps_av.tile([C, 512], F32, name="pdbc", tag="avdbc")
                doff = (pair % 2) * 2 * N
                for h in range(H):
                    nc.tensor.matmul(
                        out=pdbc[h * HD : (h + 1) * HD, doff : doff + 2 * N],
                        lhsT=ones_sb,
                        rhs=expT[:, :, h * N : (h + 1) * N],
                    )
                if pair % 2 == 1:
                    r2 = r2_pool.tile([C, 4 * N], F32, name="r2", tag="r2")
                    if RECIP_ON_DVE:
                        # 1/d on DVE (custom op, ~18-bit) frees ACT for exp
                        nc.vector.reciprocal_approx_fast(
                            out=r2, in_=pdbc[:, 0 : 4 * N]
                        )
                    else:
                        # 1/d = exp(-ln(d)); Ln+Exp share one ACT table set
                        t_ln = r2_pool.tile(
                            [C, 4 * N], F32, name="t_ln", tag="tl"
                        )
                        nc.scalar.activation(
                            out=t_ln,
                            in_=pdbc[:, 0 : 4 * N],
                            func=mybir.ActivationFunctionType.Ln,
                        )
                        nc.scalar.activation(
                            out=r2,
                            in_=t_ln,
                            func=mybir.ActivationFunctionType.Exp,
                            scale=-1.0,
                        )
                    # ---- av + norm for the 4 windows, then one batched proj
                    g0 = w0 - 2
                    psy = ps_y.tile([C, 512], F32)
                    attT4 = attT_pool.tile([C, G4 * N], BF16)
                    for kk in range(2):
                        ep = expT_prev if kk == 0 else expT
                        pav = ps_av.tile([C, 512], F32, name="pav", tag="avdbc")
                        for k in range(2):
                            j = 2 * kk + k
                            for h in range(H):
                                nc.tensor.matmul(
                                    out=pav[
                                        h * HD : (h + 1) * HD,
                                        k * N : (k + 1) * N,
                                    ],
                                    lhsT=v4_sb[:, j, h * HD : (h + 1) * HD],
                                    rhs=ep[:, k, h * N : (h + 1) * N],
                                )
                        nc.vector.tensor_tensor(
                            out=attT4[:, kk * 2 * N : (kk + 1) * 2 * N],
                            in0=pav[:, 0 : 2 * N],
                            in1=r2[:, kk * 2 * N : (kk + 1) * 2 * N],
                            op=mybir.AluOpType.mult,
                        )
                    nc.tensor.matmul(
                        out=psy[:, 0 : G4 * N],
                        lhsT=pw_sb,
                        rhs=attT4,
                    )
                    # bias add during PSUM->SBUF staging, then DMA out per 8
                    if (g0 // G4) % 2 == 0:
                        ystage = ystage_pool.tile([C, YB * N], BF16)
                    yoff = ((g0 // G4) % 2) * G4 * N
                    nc.scalar.activation(
                        out=ystage[:, yoff : yoff + G4 * N],
                        in_=psy[:, 0 : G4 * N],
                        func=mybir.ActivationFunctionType.Identity,
                        bias=pb_sb,
                    )
                    if (g0 + G4) % YB == 0:
                        blk = g0 // YB
                        nc.sync.dma_start(
                            out=out[:, blk * YB * N : (blk + 1) * YB * N],
                            in_=ystage,
                        )
                expT_prev = expT
    if split_waits:
        _split_sync_waits(nc)
    return nc


def _get_program():
    global _PROGRAM
    if _PROGRAM is None:
        _PROGRAM = _build_program()
    return _PROGRAM


# ------------------------------------------------------------------- kernel
def _core_instance_bidx(c):
    """B_ indices for core c's 128 window-instances, in device order."""
    w = np.arange(NI)
    return 512 * (w % 2) + NJ * c + (w // 2)


def _prepare_in_maps(x, y, mask, qkv_w, rpb_table, proj_w, proj_b):
    x = np.asarray(x, dtype=np.float32)
    y = np.asarray(y, dtype=np.float32)
    mask = np.asarray(mask, dtype=np.float32)
    qkv_w = np.asarray(qkv_w, dtype=np.float32)
    rpb_table = np.asarray(rpb_table, dtype=np.float32)
    proj_w = np.asarray(proj_w, dtype=np.float32)
    proj_b = np.asarray(proj_b, dtype=np.float32)

    scale = float(HD) ** -0.5

    # emb[wg, h, tq, tk] = exp(mask[wg, tq, tk] + bias[h, tq, tk])
    bias = rpb_table[REL_IDX.reshape(-1)].reshape(N, N, H).transpose(2, 0, 1)
    emb_all = np.exp(mask[:, None, :, :] + bias[None, :, :, :])
    # device layout [tk, wg, h*98+tq]
    emb_t = np.ascontiguousarray(emb_all.transpose(3, 0, 1, 2)).reshape(
        N, NW, HB
    )

    wq_h = np.ascontiguousarray((scale * qkv_w[0:C]).T).astype(NPBF16)
    wk_h = np.ascontiguousarray(qkv_w[C : 2 * C].T).astype(NPBF16)
    wv_h = np.ascontiguousarray(qkv_w[2 * C : 3 * C].T).astype(NPBF16)
    pw_h = np.ascontiguousarray(proj_w.T).astype(NPBF16)
    pb_h = np.ascontiguousarray(proj_b.reshape(C, 1)).astype(np.float32)

    in_maps = []
    bidx = []
    for c in range(NCORES):
        bi = _core_instance_bidx(c)
        bidx.append(bi)
        xc = x[bi].reshape(T, C)
        yc = y[bi].reshape(T, C)
        emb_c = np.ascontiguousarray(
            emb_t[:, NJ * c : NJ * (c + 1), :]
        ).astype(NPBF16)
        in_maps.append(
            {
                "xT": np.ascontiguousarray(xc.T).astype(NPBF16),
                "yT": np.ascontiguousarray(yc.T).astype(NPBF16),
                "emb": emb_c,
                "wq": wq_h,
                "wk": wk_h,
                "wv": wv_h,
                "pw": pw_h,
                "pb": pb_h,
            }
        )
    return in_maps, bidx


def kernel(x, y, mask, qkv_w, rpb_table, proj_w, proj_b):
    in_maps, bidx = _prepare_in_maps(
        x, y, mask, qkv_w, rpb_table, proj_w, proj_b
    )
    nc = _get_program()
    res = run_bass_kernel_spmd(nc, in_maps, list(range(NCORES)))

    out_full = np.empty((BWIN, N, C), dtype=np.float32)
    for c in range(NCORES):
        yt_o = np.asarray(res.results[c]["yT_out"]).astype(np.float32)
        out_full[bidx[c]] = yt_o.T.reshape(NI, N, C)
    return out_full



# revision 31
# speedup vs baseline: 1.5928x; 1.5928x over previous
"""Bass/Trainium2 kernel for nn_CrossWindowAttention3D (8-core SPMD).

Strategy (hardcoded for shapes B_=1024, N=98, C=96, H=3, NW=512):
- Shard 1024 window-instances over 8 cores: core c owns distinct windows
  [64c, 64c+64) for both batch replicas, interleaved (b0,j),(b1,j) so the
  exp(mask+bias) table for window j is loaded once per pair.
- Host precomputes channel-major bf16 transposes of x/y, folds the qk scale
  into the Q weights, and merges mask + relative-position bias into a single
  multiplicative table emb = exp(mask + bias) so the device softmax is
  exp(qk) * emb with no additive masking pass.
- Device computes, per window: single-matmul qT/kT projections (batched
  over 4 windows; q is scattered into a zero-initialized per-head-masked
  [C, H, 4N] layout by three partition-sliced DVE casts, so qk runs as one
  full-K matmul per window), token-major v, exp on ACT,
  one multiply by emb, unnormalized head outputs + softmax denominators via
  col-tiled matmuls (an all-ones stationary broadcasts the denominators to
  all 96 channel partitions), reciprocal on DVE, one normalize multiply, and
  a channel-major projection with bias applied during the PSUM->SBUF copy.
- Output is returned channel-major [96, 12544] per core; host transposes.
"""

import sys

sys.path.insert(0, "/opt/trn_rl_repo")

import numpy as np
import ml_dtypes

import concourse.bass as bass
import concourse.tile as tile
from concourse import mybir
from concourse.vector_clock import ScopedClock
from concourse.bass_utils import run_bass_kernel_spmd

BF16 = mybir.dt.bfloat16
F32 = mybir.dt.float32
NPBF16 = ml_dtypes.bfloat16

WS = (2, 7, 7)
N = 98            # tokens per window
C = 96            # embed dim
H = 3             # heads
HD = 32           # head dim
NW = 512          # distinct windows
BWIN = 1024       # window-instances total
NCORES = 8
NI = 128          # instances per core
NJ = 64           # distinct windows per core
T = NI * N        # tokens per core = 12544
HB = H * N        # 294


# ---------------------------------------------------------------- tile patch
def _patch_tile_tail_drain():
    """This neuronxcc build rejects >1 sync wait on CTRL-class (Drain)
    instructions; split the TileContext tail-drain waits across NOPs."""
    if getattr(tile.TileContext, "_drain_patch_applied", False):
        return

    def _drain_and_barrier_split(self, tick_clock, wait_clock):
        nc = self.nc
        carrier = nc.sync.nop(nofuse=True)
        wait_clock.add_sem_waits(
            carrier.ins, ScopedClock({None: tick_clock.global_clock})
        )
        si = carrier.ins.sync_info
        waits = list(si.on_wait or []) if si is not None else []
        if len(waits) > 1:
            si.on_wait = waits[:1]
            for w in waits[1:]:
                extra = nc.sync.nop(nofuse=True)
                esi = extra.ins.sync_info
                if esi is None:
                    extra.ins.sync_info = mybir.SyncInfo(
                        on_wait=[w], on_update=[]
                    )
                else:
                    esi.on_wait = list(esi.on_wait or []) + [w]
        nc.sync.drain()
        nc.all_engine_barrier()
        assert self.sems is not None
        popped = nc._tile_sem_poison_stack.pop()
        assert popped is self._sem_poison
        nc.clear_and_free_semaphores(list(self.sems.allocated().values()))
        nc.all_engine_barrier()

    tile.TileContext._drain_and_barrier = _drain_and_barrier_split
    tile.TileContext._drain_patch_applied = True


def _split_sync_waits(nc, max_waits=1):
    """This neuronxcc build accepts at most one sync wait per instruction.
    Hoist excess waits onto same-engine NOPs inserted just before the
    instruction (the sequencer blocks on them in order; AND-semantics of
    multiple waits is preserved)."""
    ctr = 0
    for bb in nc.main_func.blocks:
        new_list = []
        changed = False
        for inst in bb.instructions:
            si = inst.sync_info
            waits = list(si.on_wait or []) if si is not None else []
            if len(waits) > max_waits:
                si.on_wait = waits[: max_waits]
                for w in waits[max_waits:]:
                    nop = mybir.InstNoOp(
                        name=f"I-waitsplit-{ctr}", ins=[], outs=[]
                    )
                    ctr += 1
                    nop.engine = inst.engine
                    nop.sync_info = mybir.SyncInfo(on_wait=[w], on_update=[])
                    new_list.append(nop)
                changed = True
            new_list.append(inst)
        if changed:
            bb.instructions = new_list


# ------------------------------------------------------------- host helpers
def _relative_position_index():
    ws = WS
    coords = np.stack(
        np.meshgrid(
            np.arange(ws[0]), np.arange(ws[1]), np.arange(ws[2]), indexing="ij"
        )
    )
    cf = coords.reshape(3, -1)
    rel = cf[:, :, None] - cf[:, None, :]
    rel = rel.transpose(1, 2, 0).astype(np.int64)
    rel[..., 0] += ws[0] - 1
    rel[..., 1] += ws[1] - 1
    rel[..., 2] += ws[2] - 1
    rel[..., 0] *= (2 * ws[1] - 1) * (2 * ws[2] - 1)
    rel[..., 1] *= 2 * ws[2] - 1
    return rel.sum(-1)  # (N, N)


REL_IDX = _relative_position_index()


# ------------------------------------------------------------ device program
_PROGRAM = None

# tiling knobs
XCH = 32          # instances per x/y SBUF chunk (4 chunks)
ECH = 8           # emb pairs per SBUF chunk (8 chunks)
G4 = 4            # instances per q/k projection batch & proj psum batch
YB = 8            # instances per output staging buffer / DMA
EMB_DVE_HEADS = 1  # emb-mult heads handled by DVE; rest go to GPSIMD
RECIP_ON_DVE = False  # custom-DVE op fails walrus codegen (ISA wrong length)
QK_ROW_TILED = True   # per-head qk with 32-row PE tiles; no masked-q layout
EXP_PAIR = True       # one ACT exp over both pair instances (2-bank psum)
BIAS_ON_DVE = False    # proj bias-add via DVE TT instead of ACT identity


def _build_program(split_waits=True, n_pairs=NI // 2):
    _patch_tile_tail_drain()
    nc = bass.Bass()

    xT = nc.declare_dram_parameter("xT", [C, T], BF16, isOutput=False)
    yT = nc.declare_dram_parameter("yT", [C, T], BF16, isOutput=False)
    emb = nc.declare_dram_parameter("emb", [N, NJ, HB], BF16, isOutput=False)
    # plain scaled q weights; the per-head mask is applied by the
    # partition-sliced PSUM->SBUF casts into zeroed [C, H, 4N] buffers,
    # so qk stays one full-K matmul per window.
    wq = nc.declare_dram_parameter("wq", [C, C], BF16, isOutput=False)
    wk = nc.declare_dram_parameter("wk", [C, C], BF16, isOutput=False)
    wv = nc.declare_dram_parameter("wv", [C, C], BF16, isOutput=False)
    pw = nc.declare_dram_parameter("pw", [C, C], BF16, isOutput=False)
    pb = nc.declare_dram_parameter("pb", [C, 1], F32, isOutput=False)
    out = nc.declare_dram_parameter("yT_out", [C, T], BF16, isOutput=True)

    from contextlib import ExitStack

    with tile.TileContext(nc) as tc:
        with ExitStack() as ctx:
            singles = ctx.enter_context(tc.tile_pool(name="singles", bufs=1))
            xt_pool = ctx.enter_context(tc.tile_pool(name="xt", bufs=2))
            yt_pool = ctx.enter_context(tc.tile_pool(name="yt", bufs=2))
            emb_pool = ctx.enter_context(tc.tile_pool(name="emb", bufs=2))
            qt_pool = ctx.enter_context(tc.tile_pool(name="qt", bufs=3))
            qsb_pool = ctx.enter_context(tc.tile_pool(name="qsb", bufs=2))
            kt_pool = ctx.enter_context(tc.tile_pool(name="kt", bufs=3))
            v_pool = ctx.enter_context(tc.tile_pool(name="v", bufs=3))
            exp_pool = ctx.enter_context(tc.tile_pool(name="exp", bufs=3))
            expT_pool = ctx.enter_context(tc.tile_pool(name="expT", bufs=4))
            r2_pool = ctx.enter_context(tc.tile_pool(name="r2", bufs=3))
            attT_pool = ctx.enter_context(tc.tile_pool(name="attT", bufs=4))
            ystage_pool = ctx.enter_context(
                tc.tile_pool(name="ystage", bufs=2)
            )
            ps_qmk = ctx.enter_context(
                tc.tile_pool(name="ps_qmk", bufs=1, space="PSUM")
            )
            ps_v = ctx.enter_context(
                tc.tile_pool(name="ps_v", bufs=1, space="PSUM")
            )
            ps_qk = ctx.enter_context(
                tc.tile_pool(name="ps_qk", bufs=1, space="PSUM")
            )
            ps_av = ctx.enter_context(
                tc.tile_pool(name="ps_av", bufs=2, space="PSUM")
            )
            ps_y = ctx.enter_context(
                tc.tile_pool(name="ps_y", bufs=1, space="PSUM")
            )
            wq_sb = singles.tile([C, C], BF16)
            nc.sync.dma_start(out=wq_sb, in_=wq[:, :])
            wk_sb = singles.tile([C, C], BF16)
            nc.sync.dma_start(out=wk_sb, in_=wk[:, :])
            wv_sb = singles.tile([C, C], BF16)
            nc.sync.dma_start(out=wv_sb, in_=wv[:, :])
            pw_sb = singles.tile([C, C], BF16)
            nc.sync.dma_start(out=pw_sb, in_=pw[:, :])
            pb_sb = singles.tile([C, 1], F32)
            nc.sync.dma_start(out=pb_sb, in_=pb[:, :])
            ones_sb = singles.tile([N, HD], BF16)
            nc.vector.memset(ones_sb, 1.0)

            def load_xy(ch, split=False):
                """Load x/y chunk ch; split=True issues two half DMAs so
                the first consumers start as soon as the first half lands."""
                xt_c = xt_pool.tile([C, XCH * N], BF16, name="xt_c")
                yt_c = yt_pool.tile([C, XCH * N], BF16, name="yt_c")
                base = ch * XCH * N
                hw = XCH * N // 2
                for t, src in ((xt_c, xT), (yt_c, yT)):
                    if split:
                        nc.sync.dma_start(
                            out=t[:, 0:hw], in_=src[:, base : base + hw]
                        )
                        nc.sync.dma_start(
                            out=t[:, hw:],
                            in_=src[:, base + hw : base + XCH * N],
                        )
                    else:
                        nc.sync.dma_start(
                            out=t, in_=src[:, base : base + XCH * N]
                        )
                return xt_c, yt_c

            def load_emb(ek, split=False):
                # (pj, h) flattened in dim 1; trailing 1-dim broadcasts over
                # the pair instances in the emb multiply
                e = emb_pool.tile([N, ECH * H, 1, N], BF16, name="emb_c")
                eh = ECH // 2
                if split:
                    nc.sync.dma_start(
                        out=e[:, 0 : eh * H, :, :],
                        in_=emb[:, ek * ECH : ek * ECH + eh, :],
                    )
                    nc.sync.dma_start(
                        out=e[:, eh * H :, :, :],
                        in_=emb[:, ek * ECH + eh : (ek + 1) * ECH, :],
                    )
                else:
                    nc.sync.dma_start(
                        out=e, in_=emb[:, ek * ECH : (ek + 1) * ECH, :]
                    )
                return e

            # prefetch: chunk 0 split for fast start; later chunks are
            # issued halfway through the previous chunk's pairs.
            PXCH = XCH // 2          # pairs per x/y chunk
            xy_next = load_xy(0, split=True)
            emb_next = load_emb(0, split=True)
            xt_ch = yt_ch = emb_ch = None
            q_sb = kt_g = psy = ystage = None

            for pair in range(n_pairs):
                w0 = 2 * pair
                if w0 % XCH == 0:
                    xt_ch, yt_ch = xy_next
                if (
                    pair % PXCH == PXCH // 2
                    and pair // PXCH + 1 < n_pairs // PXCH
                ):
                    xy_next = load_xy(pair // PXCH + 1)
                if pair % ECH == 0:
                    emb_ch = emb_next
                if (
                    pair % ECH == ECH // 2
                    and pair // ECH + 1 < n_pairs // ECH
                ):
                    emb_next = load_emb(pair // ECH + 1)

                # q / k projections per pair, sharing one psum bank; plain
                # DVE casts evacuate them (per-head qk runs as row-tiled
                # matmuls over 32-partition slices, no masked-q layout).
                poff = (w0 % XCH) * N
                pqm = ps_qmk.tile([C, 2, 256], F32, name="pqm")
                nc.tensor.matmul(
                    out=pqm[:, 0, 0 : 2 * N],
                    lhsT=wq_sb,
                    rhs=yt_ch[:, poff : poff + 2 * N],
                )
                nc.tensor.matmul(
                    out=pqm[:, 1, 0 : 2 * N],
                    lhsT=wk_sb,
                    rhs=xt_ch[:, poff : poff + 2 * N],
                )
                q_sb = qsb_pool.tile([C, 2 * N], BF16, name="q_sb")
                nc.vector.tensor_copy(q_sb, pqm[:, 0, 0 : 2 * N])
                kt_g = kt_pool.tile([C, 2 * N], BF16)
                nc.vector.tensor_copy(kt_g, pqm[:, 1, 0 : 2 * N])

                # ---- v projections, batched 4 windows per psum tile/copy
                if w0 % G4 == 0:
                    pv4 = ps_v.tile([N, G4, 128], F32)
                    for j in range(G4):
                        col = ((w0 + j) % XCH) * N
                        nc.tensor.matmul(
                            out=pv4[:, j, 0:C],
                            lhsT=xt_ch[:, col : col + N],
                            rhs=wv_sb,
                        )
                    v4_sb = v_pool.tile([N, G4, C], BF16)
                    nc.vector.tensor_copy(v4_sb, pv4[:, :, 0:C])

                # ---- qk logits: per-head row-tiled matmuls (K=32 slices of
                # the PE array, partition groups 0/32/64). Row tiles must
                # write DISTINCT psum banks -> bank h holds head h for both
                # pair instances; one batched exp covers the whole pair.
                # Layout is (h, k, tq) from here on.
                exp_pair = exp_pool.tile([N, H, 2, N], BF16)
                pqk3 = ps_qk.tile([N, H, 512], F32)
                for k in range(2):
                    i2 = k * N
                    for h in range(H):
                        hs = slice(h * HD, (h + 1) * HD)
                        nc.tensor.matmul(
                            out=pqk3[:, h, i2 : i2 + N],
                            lhsT=kt_g[hs, i2 : i2 + N],
                            rhs=q_sb[hs, i2 : i2 + N],
                        )
                nc.scalar.activation(
                    out=exp_pair,
                    in_=pqk3[:, :, 0 : 2 * N],
                    func=mybir.ActivationFunctionType.Exp,
                )

                # ---- one multiply by emb for the pair (same distinct window)
                # head-split between DVE and GPSIMD to balance engine load
                pj = pair % ECH
                expT = expT_pool.tile([N, H, 2, N], BF16)
                eh0 = EMB_DVE_HEADS
                if eh0 > 0:
                    nc.vector.tensor_tensor(
                        out=expT[:, 0:eh0, :, :],
                        in0=exp_pair[:, 0:eh0, :, :],
                        in1=emb_ch[
                            :, pj * H : pj * H + eh0, :, :
                        ].broadcast_to((N, eh0, 2, N)),
                        op=mybir.AluOpType.mult,
                    )
                if eh0 < H:
                    nc.gpsimd.tensor_tensor(
                        out=expT[:, eh0:H, :, :],
                        in0=exp_pair[:, eh0:H, :, :],
                        in1=emb_ch[
                            :, pj * H + eh0 : (pj + 1) * H, :, :
                        ].broadcast_to((N, H - eh0, 2, N)),
                        op=mybir.AluOpType.mult,
                    )

                # ---- denominators: one [C, 4N] psum tile per 4 windows
                if pair % 2 == 0:
                    pdbc = ps_av.tile([C, 512], F32, name="pdbc", tag="avdbc")
                doff = (pair % 2) * 2 * N
                for h in range(H):
                    nc.tensor.matmul(
                        out=pdbc[h * HD : (h + 1) * HD, doff : doff + 2 * N],
                        lhsT=ones_sb,
                        rhs=expT[:, h, :, :],
                    )
                if pair % 2 == 1:
                    r2 = r2_pool.tile([C, 4 * N], F32, name="r2", tag="r2")
                    if RECIP_ON_DVE:
                        # 1/d on DVE (custom op, ~18-bit) frees ACT for exp
                        nc.vector.reciprocal_approx_fast(
                            out=r2, in_=pdbc[:, 0 : 4 * N]
                        )
                    else:
                        # 1/d = exp(-ln(d)); Ln+Exp share one ACT table set
                        t_ln = r2_pool.tile(
                            [C, 4 * N], F32, name="t_ln", tag="tl"
                        )
                        nc.scalar.activation(
                            out=t_ln,
                            in_=pdbc[:, 0 : 4 * N],
                            func=mybir.ActivationFunctionType.Ln,
                        )
                        nc.scalar.activation(
                            out=r2,
                            in_=t_ln,
                            func=mybir.ActivationFunctionType.Exp,
                            scale=-1.0,
                        )
                    # ---- av + norm for the 4 windows, then one batched proj
                    g0 = w0 - 2
                    psy = ps_y.tile([C, 512], F32)
                    attT4 = attT_pool.tile([C, G4 * N], BF16)
                    for kk in range(2):
                        ep = expT_prev if kk == 0 else expT
                        pav = ps_av.tile([C, 512], F32, name="pav", tag="avdbc")
                        for k in range(2):
                            j = 2 * kk + k
                            for h in range(H):
                                nc.tensor.matmul(
                                    out=pav[
                                        h * HD : (h + 1) * HD,
                                        k * N : (k + 1) * N,
                                    ],
                                    lhsT=v4_sb[:, j, h * HD : (h + 1) * HD],
                                    rhs=ep[:, h, k, :],
                                )
                        nc.vector.tensor_tensor(
                            out=attT4[:, kk * 2 * N : (kk + 1) * 2 * N],
                            in0=pav[:, 0 : 2 * N],
                            in1=r2[:, kk * 2 * N : (kk + 1) * 2 * N],
                            op=mybir.AluOpType.mult,
                        )
                    nc.tensor.matmul(
                        out=psy[:, 0 : G4 * N],
                        lhsT=pw_sb,
                        rhs=attT4,
                    )
                    # bias add during PSUM->SBUF staging, then DMA out per 8
                    if (g0 // G4) % 2 == 0:
                        ystage = ystage_pool.tile([C, YB * N], BF16)
                    yoff = ((g0 // G4) % 2) * G4 * N
                    if BIAS_ON_DVE:
                        nc.vector.tensor_tensor(
                            out=ystage[:, yoff : yoff + G4 * N],
                            in0=psy[:, 0 : G4 * N],
                            in1=pb_sb[:, 0:1].broadcast_to((C, G4 * N)),
                            op=mybir.AluOpType.add,
                        )
                    else:
                        nc.scalar.activation(
                            out=ystage[:, yoff : yoff + G4 * N],
                            in_=psy[:, 0 : G4 * N],
                            func=mybir.ActivationFunctionType.Identity,
                            bias=pb_sb,
                        )
                    if (g0 + G4) % YB == 0:
                        blk = g0 // YB
                        nc.sync.dma_start(
                            out=out[:, blk * YB * N : (blk + 1) * YB * N],
                            in_=ystage,
                        )
                expT_prev = expT
    if split_waits:
        _split_sync_waits(nc)
    return nc


def _get_program():
    global _PROGRAM
    if _PROGRAM is None:
        _PROGRAM = _build_program()
    return _PROGRAM


# ------------------------------------------------------------------- kernel
def _core_instance_bidx(c):
    """B_ indices for core c's 128 window-instances, in device order."""
    w = np.arange(NI)
    return 512 * (w % 2) + NJ * c + (w // 2)


def _prepare_in_maps(x, y, mask, qkv_w, rpb_table, proj_w, proj_b):
    x = np.asarray(x, dtype=np.float32)
    y = np.asarray(y, dtype=np.float32)
    mask = np.asarray(mask, dtype=np.float32)
    qkv_w = np.asarray(qkv_w, dtype=np.float32)
    rpb_table = np.asarray(rpb_table, dtype=np.float32)
    proj_w = np.asarray(proj_w, dtype=np.float32)
    proj_b = np.asarray(proj_b, dtype=np.float32)

    scale = float(HD) ** -0.5

    # emb[wg, h, tq, tk] = exp(mask[wg, tq, tk] + bias[h, tq, tk])
    bias = rpb_table[REL_IDX.reshape(-1)].reshape(N, N, H).transpose(2, 0, 1)
    emb_all = np.exp(mask[:, None, :, :] + bias[None, :, :, :])
    # device layout [tk, wg, h*98+tq]
    emb_t = np.ascontiguousarray(emb_all.transpose(3, 0, 1, 2)).reshape(
        N, NW, HB
    )

    wq_h = np.ascontiguousarray((scale * qkv_w[0:C]).T).astype(NPBF16)
    wk_h = np.ascontiguousarray(qkv_w[C : 2 * C].T).astype(NPBF16)
    wv_h = np.ascontiguousarray(qkv_w[2 * C : 3 * C].T).astype(NPBF16)
    pw_h = np.ascontiguousarray(proj_w.T).astype(NPBF16)
    pb_h = np.ascontiguousarray(proj_b.reshape(C, 1)).astype(np.float32)

    in_maps = []
    bidx = []
    for c in range(NCORES):
        bi = _core_instance_bidx(c)
        bidx.append(bi)
        xc = x[bi].reshape(T, C)
        yc = y[bi].reshape(T, C)
        emb_c = np.ascontiguousarray(
            emb_t[:, NJ * c : NJ * (c + 1), :]
        ).astype(NPBF16)
        in_maps.append(
            {
                "xT": np.ascontiguousarray(xc.T).astype(NPBF16),
                "yT": np.ascontiguousarray(yc.T).astype(NPBF16),
                "emb": emb_c,
                "wq": wq_h,
                "wk": wk_h,
                "wv": wv_h,
                "pw": pw_h,
                "pb": pb_h,
            }
        )
    return in_maps, bidx


def kernel(x, y, mask, qkv_w, rpb_table, proj_w, proj_b):
    in_maps, bidx = _prepare_in_maps(
        x, y, mask, qkv_w, rpb_table, proj_w, proj_b
    )
    nc = _get_program()
    res = run_bass_kernel_spmd(nc, in_maps, list(range(NCORES)))

    out_full = np.empty((BWIN, N, C), dtype=np.float32)
    for c in range(NCORES):
        yt_o = np.asarray(res.results[c]["yT_out"]).astype(np.float32)
        out_full[bidx[c]] = yt_o.T.reshape(NI, N, C)
    return out_full

